# revision 1
# baseline (speedup 1.0000x reference)
"""Decode-step GQA attention (bs=32, seq=1, 32 q heads / 8 kv heads, hd=128,
dim=4096, kv cache 2048) for 8 Trainium2 NeuronCores.

Sharding: tensor-parallel over heads. Core c owns kv head c and q heads
4c..4c+3: wq/wk/wv column-sharded, wo row-sharded, KV cache sharded on the
head axis. Each core computes a partial output projection; the host sums the
8 partials (no device collectives needed).

Memory-traffic design (HBM-bound): V cache and the first 3/4 of the K cache
in fp8 E3M4, the rest of K in bf16, everything else fp16. The PE accepts
mixed-dtype matmuls, so fp8 tiles feed matmuls directly against fp16
operands. Measured absmax-relative error ~1.4e-2 vs the fp32 reference
(gate 2e-2).

Instruction-count design (the PE instruction issue rate, not FLOPs, was the
previous bottleneck): scores are computed as [4b+h, seq] — one matmul per
(batch, 512-seq-quarter) with the 4 heads as the stationary free dim — so QK
is 128 matmuls instead of 512. Softmax runs along the free axis (exp on all
128 partitions, denominators via free-axis tensor_reduce, normalization by a
[128,1] per-partition broadcast). Probs are then transposed chunk-wise via
the PE, and PV is packed 4 batches per matmul (lhsT = probsT[:, 16 cols],
rhs = [v_b0|..|v_b3]); each output row uses only its own batch's 128-column
block, which plain DVE copies extract. The cache append is handled by
overwriting the stale position's K column with the roped new-token K (its
score then lands in the scores matrix automatically), zeroing the stale
row of the transposed probs, and adding the rank-1 p_new x v_new term via
one masked matmul.
"""

import functools
import sys

import numpy as np

sys.path.insert(0, "/opt/trn_rl_repo")

import concourse.bass as bass  # noqa: E402
import concourse.tile as tile  # noqa: E402
from concourse import mybir  # noqa: E402
from concourse.bass_utils import run_bass_kernel_spmd  # noqa: E402

N_HEADS = 32
N_KV_HEADS = 8
HD = 128
DIM = 4096
BS = 32
MAXSEQ = 2048
NCORES = 8
HPC = N_HEADS // NCORES  # q heads per core (4)
QW = HPC * HD  # per-core wq width (512)
SCALE = 1.0 / float(np.sqrt(np.float32(HD)))
QSEQ = 512  # seq positions per score-quarter (one PSUM bank of f32)

f32 = mybir.dt.float32
bf16 = mybir.dt.bfloat16
f16 = mybir.dt.float16
f8 = mybir.dt.float8e3


def _split_fat_waits(nc, max_waits=1):
    """walrus only encodes one semaphore wait per instruction; hoist extras
    onto preceding same-engine nops."""
    for f in nc.m.functions:
        for bb in f.blocks:
            new_list = []
            for ins in bb.instructions:
                si = ins.sync_info
                w = list(si.on_wait) if si and si.on_wait else []
                if len(w) > max_waits and ins.engine != mybir.EngineType.Unassigned:
                    extras, keep = w[:-max_waits], w[-max_waits:]
                    k = 0
                    while extras:
                        chunk, extras = extras[:max_waits], extras[max_waits:]
                        nop = mybir.InstNoOp(name=f"{ins.name}-wsplit{k}")
                        nop.engine = ins.engine
                        nop.sync_info = mybir.SyncInfo(on_wait=chunk, on_update=[])
                        new_list.append(nop)
                        k += 1
                    ins.sync_info.on_wait = keep
                new_list.append(ins)
            bb.instructions = new_list


def _seq_split(start_pos):
    """Quarter layout: NQ8 fp8 quarters then NQ16 bf16 quarters covering S."""
    S = start_pos + 1
    NCH = (S + 127) // 128  # 128-chunks
    NQ = (NCH + 3) // 4  # 512-quarters (last may be partial)
    NQ8 = (3 * NCH // 4) // 4  # whole fp8 quarters (the rest bf16)
    return S, NCH, NQ, NQ8


def _build(start_pos, reps=1):
    S, NCH, NQ, NQ8 = _seq_split(start_pos)
    NQ16 = NQ - NQ8
    S8 = QSEQ * NQ8
    NKCH = DIM // 128  # contraction chunks for the projections (32)
    LC = start_pos // 128  # chunk holding the appended position
    LP = start_pos % 128  # row within that chunk

    nc = bass.Bass()
    xT = nc.declare_dram_parameter("xT", [128, NKCH, BS], f16, isOutput=False)
    wqkv = nc.declare_dram_parameter("wqkv", [128, NKCH, QW + 2 * HD], f16, isOutput=False)
    wo = nc.declare_dram_parameter("wo", [128, HPC, DIM], f16, isOutput=False)
    kT8 = nc.declare_dram_parameter("kT8", [128, max(NQ8, 1), BS, QSEQ], f8, isOutput=False)
    kT16 = nc.declare_dram_parameter("kT16", [128, max(NQ16, 1), BS, QSEQ], bf16, isOutput=False)
    v8 = nc.declare_dram_parameter("v8", [128, MAXSEQ // 128, BS, HD], f8, isOutput=False)
    cosq = nc.declare_dram_parameter("cosq", [BS, QW], f16, isOutput=False)
    sinq = nc.declare_dram_parameter("sinq", [BS, QW], f16, isOutput=False)
    cosk = nc.declare_dram_parameter("cosk", [BS, HD], f16, isOutput=False)
    sink = nc.declare_dram_parameter("sink", [BS, HD], f16, isOutput=False)
    iden = nc.declare_dram_parameter("iden", [128, 128], f32, isOutput=False)
    iden16 = nc.declare_dram_parameter("iden16", [128, 128], f16, isOutput=False)
    # column permutation (32h + b) -> (4b + h) applied during the probs
    # transposes, so probsT quad slices are contiguous single-dim APs
    perm16 = nc.declare_dram_parameter("perm16", [128, 128], f16, isOutput=False)
    # rowmask[p] = 0.0 at the stale row LP, 1.0 elsewhere
    rowmask = nc.declare_dram_parameter("rowmask", [128, 1], f16, isOutput=False)
    # iden4[b, 32h + b'] = 1.0 if b' == b (new-token scatter mask)
    iden4 = nc.declare_dram_parameter("iden4", [BS, 128], f32, isOutput=False)
    out = nc.declare_dram_parameter("out", [BS, DIM], f16, isOutput=True)

    with tile.TileContext(nc) as tc:
        with (
            tc.tile_pool(name="const", bufs=1) as const,
            tc.tile_pool(name="wpool", bufs=2) as wpool,
            tc.tile_pool(name="ktpool", bufs=2) as ktpool,
            tc.tile_pool(name="kt16pool", bufs=1) as kt16pool,
            tc.tile_pool(name="vpool", bufs=1) as vpool,
            tc.tile_pool(name="sm", bufs=1) as smpool,
            tc.tile_pool(name="wopool", bufs=1) as wopool,
            tc.tile_pool(name="outpool", bufs=1) as outpool,
        ):
            # ---- constants ----
            iden_sb = const.tile([128, 128], f32)
            nc.sync.dma_start(out=iden_sb[:], in_=iden[:])
            iden16_sb = const.tile([128, 128], f16)
            nc.sync.dma_start(out=iden16_sb[:], in_=iden16[:])
            perm16_sb = const.tile([128, 128], f16)
            nc.sync.dma_start(out=perm16_sb[:], in_=perm16[:])
            iden4_sb = const.tile([BS, 128], f32)
            nc.sync.dma_start(out=iden4_sb[:], in_=iden4[:])
            xT_sb = const.tile([128, NKCH, BS], f16)
            nc.sync.dma_start(out=xT_sb[:], in_=xT[:])
            cosq_sb = const.tile([BS, QW], f16)
            nc.sync.dma_start(out=cosq_sb[:], in_=cosq[:])
            sinq_sb = const.tile([BS, QW], f16)
            nc.sync.dma_start(out=sinq_sb[:], in_=sinq[:])
            cosk_sb = const.tile([BS, HD], f16)
            nc.sync.dma_start(out=cosk_sb[:], in_=cosk[:])
            sink_sb = const.tile([BS, HD], f16)
            nc.sync.dma_start(out=sink_sb[:], in_=sink[:])
            rowmask_sb = const.tile([128, 1], f16)
            nc.sync.dma_start(out=rowmask_sb[:], in_=rowmask[:])
            ones32 = const.tile([1, BS], f16)
            nc.vector.memset(ones32[:], 1.0)

            qT_all = const.tile([128, BS, HPC], f16)  # col = 4b + h
            vnew = const.tile([BS, HD], f16)
            kroT = const.tile([128, BS], f32)

            import contextlib

            rep_ctx = (
                tc.For_i(0, reps, 1, name="rep")
                if reps > 1
                else contextlib.nullcontext()
            )
            with rep_ctx:
                _emit_body(
                    nc, tc, const, wpool, ktpool, kt16pool, vpool, smpool,
                    wopool, outpool, iden_sb, iden16_sb, perm16_sb, iden4_sb,
                    rowmask_sb, xT_sb,
                    cosq_sb, sinq_sb, cosk_sb, sink_sb, ones32, qT_all, vnew,
                    kroT, wqkv, wo, kT8, kT16, v8, out,
                    S, NCH, NQ, NQ8, LC, LP, NKCH,
                )

    _split_fat_waits(nc)
    return nc


def _emit_body(
    nc, tc, const, wpool, ktpool, kt16pool, vpool, smpool, wopool, outpool,
    iden_sb, iden16_sb, perm16_sb, iden4_sb, rowmask_sb, xT_sb, cosq_sb, sinq_sb, cosk_sb, sink_sb,
    ones32, qT_all, vnew, kroT, wqkv, wo, kT8, kT16, v8, out,
    S, NCH, NQ, NQ8, LC, LP, NKCH,
):
    NQ16 = NQ - NQ8
    KVW = QW + 2 * HD  # 768

    # ---- phase 1: QKV projections ----
    with (
        tc.tile_pool(name="psum_p1", bufs=1, space="PSUM") as psum_p1,
        tc.tile_pool(name="psum_t2", bufs=2, space="PSUM") as psum_t2,
    ):
        q_ps = psum_p1.tile([BS, QW], f32)
        kv_ps = psum_p1.tile([BS, 2 * HD], f32)
        WCH = 4  # contraction chunks per wqkv DMA
        for k in range(NKCH // WCH):
            w_t = wpool.tile([128, WCH, KVW], f16)
            w_eng = nc.sync if k % 2 == 0 else nc.scalar
            w_eng.dma_start(out=w_t[:], in_=wqkv[:, WCH * k : WCH * (k + 1), :])
            for j in range(WCH):
                kk = WCH * k + j
                st = kk == 0
                sp = kk == NKCH - 1
                lhsT = xT_sb[:, kk, :]
                nc.tensor.matmul(q_ps[:], lhsT, w_t[:, j, :QW], start=st, stop=sp)
                nc.tensor.matmul(
                    kv_ps[:], lhsT, w_t[:, j, QW:], start=st, stop=sp
                )

        # ---- phase 2: rope, q/k transposes, new-token prep ----
        p2 = const
        # rope(q)
        q_sw = p2.tile([BS, QW], f32)
        q_ps3 = q_ps[:].rearrange("p (i two) -> p i two", two=2)
        q_sw3 = q_sw[:].rearrange("p (i two) -> p i two", two=2)
        nc.vector.tensor_copy(out=q_sw3[:, :, 0], in_=q_ps3[:, :, 1])
        nc.vector.tensor_copy(out=q_sw3[:, :, 1], in_=q_ps3[:, :, 0])
        q_ro = p2.tile([BS, QW], f32)
        nc.vector.tensor_tensor(q_ro[:], q_ps[:], cosq_sb[:], mybir.AluOpType.mult)
        nc.vector.tensor_tensor(q_sw[:], q_sw[:], sinq_sb[:], mybir.AluOpType.mult)
        nc.vector.tensor_tensor(q_ro[:], q_ro[:], q_sw[:], mybir.AluOpType.add)
        # rope(k) on kv_ps[:, :HD]
        k_sw = p2.tile([BS, HD], f32)
        k_ps3 = kv_ps[:, :HD].rearrange("p (i two) -> p i two", two=2)
        k_sw3 = k_sw[:].rearrange("p (i two) -> p i two", two=2)
        nc.vector.tensor_copy(out=k_sw3[:, :, 0], in_=k_ps3[:, :, 1])
        nc.vector.tensor_copy(out=k_sw3[:, :, 1], in_=k_ps3[:, :, 0])
        k_ro = p2.tile([BS, HD], f32)
        nc.vector.tensor_tensor(
            k_ro[:], kv_ps[:, :HD], cosk_sb[:], mybir.AluOpType.mult
        )
        nc.vector.tensor_tensor(k_sw[:], k_sw[:], sink_sb[:], mybir.AluOpType.mult)
        nc.vector.tensor_tensor(k_ro[:], k_ro[:], k_sw[:], mybir.AluOpType.add)
        # v_new (no rope)
        nc.vector.tensor_copy(out=vnew[:], in_=kv_ps[:, HD:])

        # q^T assembly (b-major columns): qT_all[:, b, h] = q_ro[b, 128h + :]
        for h in range(HPC):
            ps_qt = psum_t2.tile([128, BS], f32, tag="tr")
            nc.tensor.transpose(
                ps_qt[:], q_ro[:, 128 * h : 128 * (h + 1)], iden_sb[:BS, :BS]
            )
            nc.vector.tensor_copy(out=qT_all[:, :, h], in_=ps_qt[:])
        # k_ro^T [128d, 32b] for the stale-column overwrite
        ps_kt = psum_t2.tile([128, BS], f32, tag="tr")
        nc.tensor.transpose(ps_kt[:], k_ro[:], iden_sb[:BS, :BS])
        nc.vector.tensor_copy(out=kroT[:], in_=ps_kt[:])

    # ---- phase 3a: QK scores in 512-wide quarters ----
    # Matmul (and ACT) outputs must start at 32-aligned partitions, so each
    # batch's [4, 512] score tile lands at base partition 0: first in a
    # bank-rotated PSUM tile, then (DVE/ACT alternating) in a [4h, 32b, 512s]
    # staging tile, and one SBUF->SBUF DMA per quarter fans the staging out
    # to the packed layout with partition p = 32h + b.
    scores = smpool.tile([128, NQ, QSEQ], f16)
    stage = smpool.tile([HPC, BS, QSEQ], f16)

    def _copy3(i, out, in_):
        # spread PSUM->SBUF base-0 copies over the two PSUM-capable engines
        if i % 2 == 1:
            nc.scalar.activation(
                out=out, in_=in_, func=mybir.ActivationFunctionType.Copy
            )
        else:
            nc.vector.tensor_copy(out=out, in_=in_)
    with tc.tile_pool(name="ps_qk", bufs=8, space="PSUM") as psQK:
        for qi in range(NQ):
            qs = QSEQ * qi
            qw = min(QSEQ, S - qs)
            if qi < NQ8:
                kt_t = ktpool.tile([128, BS, QSEQ], f8, tag="kt")
                nc.sync.dma_start(out=kt_t[:], in_=kT8[:, qi, :, :])
            else:
                kt_t = kt16pool.tile([128, BS, QSEQ], bf16, tag="kt16")
                nc.sync.dma_start(out=kt_t[:], in_=kT16[:, qi - NQ8, :, :])
            if qs <= S - 1 < qs + QSEQ:
                # overwrite the stale (appended) position's K with rope(k_new)
                nc.vector.tensor_copy(out=kt_t[:, :, S - 1 - qs], in_=kroT[:])
            for b in range(BS):
                ps_s = psQK.tile([HPC, QSEQ], f32, tag="qk", bufs=8)
                nc.tensor.matmul(
                    ps_s[:, :qw],
                    qT_all[:, b, :],
                    kt_t[:, b, :qw],
                    start=True,
                    stop=True,
                )
                _copy3(b, stage[:, b, :qw], ps_s[:, :qw])
            nc.sync.dma_start(out=scores[:, qi, :], in_=stage[:])

    # ---- phase 3b: softmax along free axis (p = 32h + b) ----
    probs = smpool.tile([128, NQ * QSEQ], f16)
    nc.scalar.activation(
        out=probs[:],
        in_=scores[:].rearrange("p a b -> p (a b)"),
        func=mybir.ActivationFunctionType.Exp,
        scale=SCALE,
    )
    if S < NQ * QSEQ:  # zero the tail beyond S before reducing
        nc.vector.memset(probs[:, S:], 0.0)
    den4 = smpool.tile([128, NQ], f32)
    nc.vector.tensor_reduce(
        out=den4[:], in_=probs[:].rearrange("p (a b) -> p a b", a=NQ),
        axis=mybir.AxisListType.X, op=mybir.AluOpType.add,
    )
    den = smpool.tile([128, 1], f32)
    nc.vector.tensor_reduce(
        out=den[:], in_=den4[:], axis=mybir.AxisListType.X,
        op=mybir.AluOpType.add,
    )
    inv = smpool.tile([128, 1], f32)
    nc.vector.reciprocal(inv[:], den[:])
    nc.vector.tensor_tensor(
        probs[:],
        probs[:],
        inv[:].to_broadcast([128, NQ * QSEQ]),
        mybir.AluOpType.mult,
    )

    # ---- phase 3c: transpose probs to [s, 4b+h] chunks; new-token mask ----
    probsT = smpool.tile([128, NCH, 128], f16)
    E2 = smpool.tile([BS, 128], f16)
    with (
        tc.tile_pool(name="ps_tr", bufs=3, space="PSUM") as psT,
        tc.tile_pool(name="ps_eb", bufs=1, space="PSUM") as psEb,
    ):
        for c in range(NCH):
            cw = min(128, S - 128 * c)
            ps_p = psT.tile([128, 128], f16, tag="ptr")
            nc.tensor.transpose(
                ps_p[:cw, :], probs[:, 128 * c : 128 * c + cw], perm16_sb[:]
            )
            _copy3(c, probsT[:cw, c, :], ps_p[:cw, :])
        # E2[b, 4b'+h] = p_new[4b'+h] * (b == b'): rank-1 new-token update.
        # p_new is the probs column at the appended position; transpose it to
        # a row at base partition 0, then broadcast down 32 partitions.
        ps_pnr = psT.tile([1, 128], f16, tag="pnr")
        nc.tensor.transpose(
            ps_pnr[:], probs[:, S - 1 : S], perm16_sb[:]
        )
        pnr = smpool.tile([1, 128], f16)
        nc.vector.tensor_copy(out=pnr[:], in_=ps_pnr[:])
        ps_eb = psEb.tile([BS, 128], f32)
        nc.tensor.matmul(ps_eb[:], ones32[:], pnr[:], start=True, stop=True)
        nc.vector.tensor_tensor(E2[:], ps_eb[:], iden4_sb[:], mybir.AluOpType.mult)
        # stale row of probsT must not touch the stale cached V
        nc.vector.tensor_tensor(
            probsT[:, LC, :], probsT[:, LC, :],
            rowmask_sb[:].to_broadcast([128, 128]), mybir.AluOpType.mult,
        )

    # ---- phase 3d: PV, 4 batches per matmul ----
    # probsT columns are (4b + h); quad Q's lhsT picks batches 4Q..4Q+3 with
    # free order (r, h) so output rows are 4r + h. Each row's result sits in
    # its own batch's 128-column block (the rest is discarded): quads are
    # copied to SBUF, each 128-block PE-transposed to [128d, 16], and the 4
    # wanted columns land in attnT via free-offset copies (engine partition
    # offsets must be 32-aligned, free offsets are unrestricted).
    attnT = smpool.tile([128, BS, HPC], f16)  # [d, 4b + h]
    attnT_f = attnT[:].rearrange("p b h -> p (b h)")
    quad_tiles = []
    with tc.tile_pool(name="ps_pv", bufs=8, space="PSUM") as psPV:
        pv_tiles = [psPV.tile([4 * HPC, 4 * HD], f32, tag="pv", bufs=8,
                              name=f"pv{i}")
                    for i in range(8)]
        for c in range(NCH):
            cw = min(128, S - 128 * c)
            if c % 4 == 0:  # V streams in 4-chunk groups, double-buffered
                v_g = vpool.tile([128, 4, BS, HD], f8, tag="v", bufs=2)
                nc.scalar.dma_start(
                    out=v_g[:, : min(4, NCH - c), :, :],
                    in_=v8[:, c : min(c + 4, NCH), :, :],
                )
                v_v = v_g[:].rearrange("p c b d -> p c (b d)")
            for Q in range(8):
                nc.tensor.matmul(
                    pv_tiles[Q][:],
                    probsT[:cw, c, 16 * Q : 16 * (Q + 1)],
                    v_v[:cw, c % 4, 512 * Q : 512 * (Q + 1)],
                    start=(c == 0),
                    stop=(c == NCH - 1),
                )
        for Q in range(8):
            quad_sb = smpool.tile([4 * HPC, 4 * HD], f16, tag="quad", bufs=8,
                                  name=f"quad{Q}")
            _copy3(Q, quad_sb[:], pv_tiles[Q][:])
            quad_tiles.append(quad_sb)
    with tc.tile_pool(name="ps_at", bufs=2, space="PSUM") as psAT:
        for Q in range(8):
            for j in range(4):
                ps_at = psAT.tile([128, 4 * HPC], f16, tag="at")
                nc.tensor.transpose(
                    ps_at[:], quad_tiles[Q][:, 128 * j : 128 * (j + 1)],
                    iden16_sb[: 4 * HPC, : 4 * HPC],
                )
                b = 4 * Q + j
                _copy3(j, attnT_f[:, HPC * b : HPC * (b + 1)],
                       ps_at[:, HPC * j : HPC * (j + 1)])

    # ---- phase 3e: new-token rank-1 term, added in [d, 4b + h] form ----
    with tc.tile_pool(name="ps_e2", bufs=1, space="PSUM") as psE2:
        ps_e2 = psE2.tile([128, 128], f32)
        nc.tensor.matmul(ps_e2[:], vnew[:], E2[:], start=True, stop=True)
        nc.vector.tensor_tensor(
            attnT_f[:], attnT_f[:], ps_e2[:], mybir.AluOpType.add
        )

    # ---- phase 4: output projection (wo streamed in 512-col chunks) ----
    NO = 8  # chunks of DIM/NO=512 (PSUM bank free-size max)
    NW = DIM // NO
    out_sb = outpool.tile([BS, DIM], f16)
    with tc.tile_pool(name="ps_o", bufs=2, space="PSUM") as psO:
        for n in range(NO):
            ns = slice(NW * n, NW * (n + 1))
            wo_n = wopool.tile([128, HPC, NW], f16, tag="wo", bufs=2)
            w_eng = nc.sync if n % 2 == 0 else nc.scalar
            w_eng.dma_start(out=wo_n[:], in_=wo[:, :, ns])
            ps_o = psO.tile([BS, NW], f32)
            for j in range(HPC):
                nc.tensor.matmul(
                    ps_o[:],
                    attnT[:, :, j],
                    wo_n[:, j, :],
                    start=(j == 0),
                    stop=(j == HPC - 1),
                )
            nc.vector.tensor_copy(out=out_sb[:, ns], in_=ps_o[:])
            nc.sync.dma_start(out=out[:, ns], in_=out_sb[:, ns])


@functools.lru_cache(maxsize=8)
def _built(start_pos, reps=1):
    return _build(start_pos, reps)


def _host_prep(x, wq, wk, wv, wo, cache_k, cache_v, freqs_cos, freqs_sin, start_pos):
    import ml_dtypes

    f8np = ml_dtypes.float8_e3m4
    bf16np = ml_dtypes.bfloat16
    S, NCH, NQ, NQ8 = _seq_split(start_pos)
    NQ16 = NQ - NQ8
    S8 = QSEQ * NQ8

    x = np.ascontiguousarray(np.asarray(x, dtype=np.float32)).reshape(BS, DIM)
    wq = np.asarray(wq, dtype=np.float32)
    wk = np.asarray(wk, dtype=np.float32)
    wv = np.asarray(wv, dtype=np.float32)
    wo = np.asarray(wo, dtype=np.float32)
    cache_k = np.asarray(cache_k, dtype=np.float32)
    cache_v = np.asarray(cache_v, dtype=np.float32)
    cos = np.asarray(freqs_cos, dtype=np.float32).reshape(HD // 2)
    sin = np.asarray(freqs_sin, dtype=np.float32).reshape(HD // 2)

    # x^T chunks: xT[p, c, b] = x[b, 128c + p]
    xT = np.ascontiguousarray(
        x.reshape(BS, DIM // 128, 128).transpose(2, 1, 0).astype(np.float16)
    )

    cosF = np.empty(HD, np.float32)
    cosF[0::2] = cos
    cosF[1::2] = cos
    sinF = np.empty(HD, np.float32)
    sinF[0::2] = -sin
    sinF[1::2] = sin
    cosq = np.ascontiguousarray(
        np.broadcast_to(np.tile(cosF, HPC), (BS, QW)).astype(np.float16))
    sinq = np.ascontiguousarray(
        np.broadcast_to(np.tile(sinF, HPC), (BS, QW)).astype(np.float16))
    cosk = np.ascontiguousarray(np.broadcast_to(cosF, (BS, HD)).astype(np.float16))
    sink = np.ascontiguousarray(np.broadcast_to(sinF, (BS, HD)).astype(np.float16))
    iden = np.eye(128, dtype=np.float32)
    iden16 = np.eye(128, dtype=np.float16)
    rowmask = np.ones((128, 1), dtype=np.float16)
    rowmask[start_pos % 128, 0] = 0.0
    perm16 = np.zeros((128, 128), dtype=np.float16)
    for h in range(HPC):
        for b in range(BS):
            perm16[32 * h + b, HPC * b + h] = 1.0
    # columns are 4b' + h: nonzero where b' == b (row), for every h
    iden4 = np.ascontiguousarray(np.repeat(np.eye(BS, dtype=np.float32), HPC, axis=1))

    # K^T quarters, padded to full QSEQ width
    SPAD = QSEQ * NQ
    in_maps = []
    for c in range(NCORES):
        wqkv_c = np.concatenate(
            [
                wq[:, QW * c : QW * (c + 1)],
                wk[:, HD * c : HD * (c + 1)],
                wv[:, HD * c : HD * (c + 1)],
            ],
            axis=1,
        )  # [DIM, 768]
        wqkv_c = np.ascontiguousarray(
            wqkv_c.reshape(DIM // 128, 128, QW + 2 * HD)
            .transpose(1, 0, 2)
            .astype(np.float16)
        )
        wo_c = np.ascontiguousarray(
            wo[QW * c : QW * (c + 1), :]
            .reshape(HPC, 128, DIM)
            .transpose(1, 0, 2)
            .astype(np.float16)
        )
        ck = np.zeros((BS, SPAD, HD), np.float32)
        ck[:, :S] = cache_k[:, :S, c, :]
        # kq[d, q, b, s'] = ck[b, 512q + s', d]
        kT8_c = np.ascontiguousarray(
            ck[:, :max(S8, 1) if NQ8 else 1]
            .reshape(BS, max(NQ8, 1), -1, HD)
            .transpose(3, 1, 0, 2)
            .astype(f8np)
        ) if NQ8 else np.zeros((128, 1, BS, QSEQ), f8np)
        kT16_c = np.ascontiguousarray(
            ck[:, S8:]
            .reshape(BS, max(NQ16, 1), -1, HD)
            .transpose(3, 1, 0, 2)
            .astype(bf16np)
        ) if NQ16 else np.zeros((128, 1, BS, QSEQ), bf16np)
        # v8[p, ch, b, d] = cache_v[b, 128ch + p, c, d]
        v8_c = np.ascontiguousarray(
            cache_v[:, :, c, :]
            .reshape(BS, MAXSEQ // 128, 128, HD)
            .transpose(2, 1, 0, 3)
            .astype(f8np)
        )
        in_maps.append(
            {
                "xT": xT,
                "wqkv": wqkv_c,
                "wo": wo_c,
                "kT8": kT8_c,
                "kT16": kT16_c,
                "v8": v8_c,
                "cosq": cosq,
                "sinq": sinq,
                "cosk": cosk,
                "sink": sink,
                "iden": iden,
                "iden16": iden16,
                "perm16": perm16,
                "rowmask": rowmask,
                "iden4": iden4,
            }
        )
    return in_maps


def kernel(
    x,
    wq,
    wk,
    wv,
    wo,
    cache_k,
    cache_v,
    freqs_cos,
    freqs_sin,
    start_pos,
    _trace=False,
    **_unused,
):
    sp = int(start_pos)
    nc = _built(sp)
    in_maps = _host_prep(
        x, wq, wk, wv, wo, cache_k, cache_v, freqs_cos, freqs_sin, sp
    )
    res = run_bass_kernel_spmd(nc, in_maps, list(range(NCORES)), trace=_trace)
    acc = np.zeros((BS, DIM), np.float32)
    for i in range(NCORES):
        acc += np.asarray(res.results[i]["out"], dtype=np.float32)
    out = acc.reshape(BS, 1, DIM)
    if _trace:
        return out, res
    return out



# revision 36
# speedup vs baseline: 1.7469x; 1.7469x over previous
"""Decode-step GQA attention (bs=32, seq=1, 32 q heads / 8 kv heads, hd=128,
dim=4096, kv cache 2048) for 8 Trainium2 NeuronCores.

Sharding: tensor-parallel over heads. Core c owns kv head c and q heads
4c..4c+3: wq/wk/wv column-sharded, wo row-sharded, KV cache sharded on the
head axis. Each core computes a partial output projection; the host sums the
8 partials (no device collectives needed).

Memory-traffic design (HBM-bound): V cache and the first 3/4 of the K cache
in fp8 E3M4, the last K quarter bf16, wqkv f16, wo fp8 E3M4 pre-scaled x64
(descaled in the output drain). ~27.6MB/core streams at the DMA roofline.

Tensor-engine design: QK packs 4 batches per PSUM bank via col-tiled matmuls
(tile_position=(0,32l)), so PSUM drains are full-width [128,512] copies
instead of narrow [4,512] ones. scores land in partition layout p=4b+h via a
two-hop DMA fan-out (single-level partition patterns only; nested ones
corrupt the DMA lowering). Softmax runs along the free axis; probs chunks
are PE-transposed with an identity (p=4b+h is already the probsT column
order). PV packs 4 batches per matmul with quads col-tiled 4-per-bank, V is
fully SBUF-resident (prefetched during the projections), and wo is
prefetched fp8 so the output projection never waits on DMA.
"""

import functools
import sys

import numpy as np

sys.path.insert(0, "/opt/trn_rl_repo")

import concourse.bass as bass  # noqa: E402
import concourse.tile as tile  # noqa: E402
from concourse import mybir  # noqa: E402
from concourse.bass_utils import run_bass_kernel_spmd  # noqa: E402

N_HEADS = 32
N_KV_HEADS = 8
HD = 128
DIM = 4096
BS = 32
MAXSEQ = 2048
NCORES = 8
HPC = N_HEADS // NCORES  # q heads per core (4)
QW = HPC * HD  # per-core wq width (512)
SCALE = 1.0 / float(np.sqrt(np.float32(HD)))
QSEQ = 512  # seq positions per score-quarter (one PSUM bank of f32)
WO_SCALE = 64.0  # host pre-scales wo by this; output drain divides it out

f32 = mybir.dt.float32
bf16 = mybir.dt.bfloat16
f16 = mybir.dt.float16
f8 = mybir.dt.float8e3


def _split_fat_waits(nc, max_waits=1):
    """walrus only encodes one semaphore wait per instruction; hoist extras
    onto preceding same-engine nops."""
    for f in nc.m.functions:
        for bb in f.blocks:
            new_list = []
            for ins in bb.instructions:
                si = ins.sync_info
                w = list(si.on_wait) if si and si.on_wait else []
                if len(w) > max_waits and ins.engine != mybir.EngineType.Unassigned:
                    extras, keep = w[:-max_waits], w[-max_waits:]
                    k = 0
                    while extras:
                        chunk, extras = extras[:max_waits], extras[max_waits:]
                        nop = mybir.InstNoOp(name=f"{ins.name}-wsplit{k}")
                        nop.engine = ins.engine
                        nop.sync_info = mybir.SyncInfo(on_wait=chunk, on_update=[])
                        new_list.append(nop)
                        k += 1
                    ins.sync_info.on_wait = keep
                new_list.append(ins)
            bb.instructions = new_list


def _seq_split(start_pos):
    """Quarter layout: NQ8 fp8 quarters then NQ16 bf16 quarters covering S."""
    S = start_pos + 1
    NCH = (S + 127) // 128  # 128-chunks
    NQ = (NCH + 3) // 4  # 512-quarters (last may be partial)
    NQ8 = (3 * NCH // 4) // 4  # whole fp8 quarters (the rest bf16)
    return S, NCH, NQ, NQ8


def _build(start_pos, reps=1):
    S, NCH, NQ, NQ8 = _seq_split(start_pos)
    NQ16 = NQ - NQ8
    NKCH = DIM // 128  # contraction chunks for the projections (32)
    LC = start_pos // 128  # chunk holding the appended position
    LP = start_pos % 128  # row within that chunk

    nc = bass.Bass()
    xT = nc.declare_dram_parameter("xT", [128, NKCH, BS], f16, isOutput=False)
    wqkv = nc.declare_dram_parameter("wqkv", [128, NKCH, QW + 2 * HD], f16, isOutput=False)
    wo = nc.declare_dram_parameter("wo", [128, HPC, DIM], f8, isOutput=False)
    kT8 = nc.declare_dram_parameter("kT8", [128, max(NQ8, 1), BS, QSEQ], f8, isOutput=False)
    kT16 = nc.declare_dram_parameter("kT16", [128, max(NQ16, 1), BS, QSEQ], bf16, isOutput=False)
    v8 = nc.declare_dram_parameter("v8", [128, MAXSEQ // 128, BS, HD], f8, isOutput=False)
    cosq = nc.declare_dram_parameter("cosq", [BS, QW], f16, isOutput=False)
    sinq = nc.declare_dram_parameter("sinq", [BS, QW], f16, isOutput=False)
    cosk = nc.declare_dram_parameter("cosk", [BS, HD], f16, isOutput=False)
    sink = nc.declare_dram_parameter("sink", [BS, HD], f16, isOutput=False)
    iden = nc.declare_dram_parameter("iden", [128, 128], f32, isOutput=False)
    iden16 = nc.declare_dram_parameter("iden16", [128, 128], f16, isOutput=False)
    # rowmask[p] = 0.0 at the stale row LP, 1.0 elsewhere
    rowmask = nc.declare_dram_parameter("rowmask", [128, 1], f16, isOutput=False)
    # iden4[b, 4b' + h] = 1.0 if b' == b (new-token scatter mask)
    iden4 = nc.declare_dram_parameter("iden4", [BS, 128], f32, isOutput=False)
    out = nc.declare_dram_parameter("out", [BS, DIM], f16, isOutput=True)

    with tile.TileContext(nc) as tc:
        with (
            tc.tile_pool(name="const", bufs=1) as const,
            tc.tile_pool(name="wpool", bufs=3) as wpool,
            tc.tile_pool(name="ktpool", bufs=1) as ktpool,
            tc.tile_pool(name="kt16pool", bufs=1) as kt16pool,
            tc.tile_pool(name="vpool", bufs=1) as vpool,
            tc.tile_pool(name="sm", bufs=1) as smpool,
            tc.tile_pool(name="wopool", bufs=1) as wopool,
            tc.tile_pool(name="outpool", bufs=1) as outpool,
        ):
            # ---- constants, striped so each queue's first bytes are the
            # proj-critical ones (DMA queues time-slice coarsely: per-queue
            # FIFO position is the only priority control we have) ----
            xT_sb = const.tile([128, NKCH, BS], f16)
            nc.sync.dma_start(out=xT_sb[:], in_=xT[:])
            cosq_sb = const.tile([BS, QW], f16)
            nc.sync.dma_start(out=cosq_sb[:], in_=cosq[:])
            iden_sb = const.tile([128, 128], f32)
            nc.sync.dma_start(out=iden_sb[:], in_=iden[:])
            iden16_sb = const.tile([128, 128], f16)
            nc.gpsimd.dma_start(out=iden16_sb[:], in_=iden16[:])
            cosk_sb = const.tile([BS, HD], f16)
            nc.gpsimd.dma_start(out=cosk_sb[:], in_=cosk[:])
            sink_sb = const.tile([BS, HD], f16)
            nc.gpsimd.dma_start(out=sink_sb[:], in_=sink[:])
            iden4_sb = const.tile([BS, 128], f32)
            nc.gpsimd.dma_start(out=iden4_sb[:], in_=iden4[:])
            rowmask_sb = const.tile([128, 1], f16)
            nc.gpsimd.dma_start(out=rowmask_sb[:], in_=rowmask[:])
            sinq_sb = const.tile([BS, QW], f16)
            nc.gpsimd.dma_start(out=sinq_sb[:], in_=sinq[:])
            ones32 = const.tile([1, BS], f16)
            nc.vector.memset(ones32[:], 1.0)
            ones1r = const.tile([1, 128], f16)
            nc.vector.memset(ones1r[:], 1.0)
            inv64b = const.tile([96, 1], f32)
            nc.vector.memset(inv64b[:], 1.0 / WO_SCALE)




            qT_all = const.tile([128, BS, HPC], f16)  # col order (b, h)
            vnew = const.tile([BS, HD], f16)
            kroT = const.tile([128, BS], f32)

            import contextlib

            rep_ctx = (
                tc.For_i(0, reps, 1, name="rep")
                if reps > 1
                else contextlib.nullcontext()
            )
            with rep_ctx:
                _emit_body(
                    nc, tc, const, wpool, ktpool, kt16pool, smpool, outpool,
                    iden_sb, iden16_sb, iden4_sb, rowmask_sb, xT_sb,
                    cosq_sb, sinq_sb, cosk_sb, sink_sb, ones32, qT_all, vnew,
                    kroT, vpool, v8, wopool, wo, wqkv, kT8, kT16, out, ones1r, inv64b,
                    S, NCH, NQ, NQ8, LC, LP, NKCH,
                )

    _split_fat_waits(nc)
    return nc


def _emit_body(
    nc, tc, const, wpool, ktpool, kt16pool, smpool, outpool,
    iden_sb, iden16_sb, iden4_sb, rowmask_sb, xT_sb, cosq_sb, sinq_sb,
    cosk_sb, sink_sb, ones32, qT_all, vnew, kroT, vpool, v8, wopool, wo, wqkv, kT8,
    kT16, out, ones1r, inv64b, S, NCH, NQ, NQ8, LC, LP, NKCH,
):
    NQ16 = NQ - NQ8
    KVW = QW + 2 * HD  # 768

    # ---- phase 1: QKV projections ----
    with (
        tc.tile_pool(name="psum_p1", bufs=1, space="PSUM") as psum_p1,
        tc.tile_pool(name="psum_t2", bufs=2, space="PSUM") as psum_t2,
    ):
        # 3-way col-tiled: contraction chunk kk accumulates into partition
        # band 32*(kk%3); the three partial sums are merged on the DVE.
        q_ps = psum_p1.tile([128, QW], f32)
        kv_ps = psum_p1.tile([128, 2 * HD], f32)
        WCH = 4  # contraction chunks per wqkv DMA
        w_tiles = []
        for k in range(NKCH // WCH):
            w_t = wpool.tile([128, WCH, KVW], f16, tag="w")
            nc.sync.dma_start(out=w_t[:], in_=wqkv[:, WCH * k : WCH * (k + 1), :])
            w_tiles.append(w_t)
        NP3 = NKCH // 3  # chunks per position band: 0,1 have 11, band 2 has 10
        for k in range(NKCH // WCH):
            w_t = w_tiles[k]
            for j in range(WCH):
                kk = WCH * k + j
                po = 32 * (kk % 3)
                st = kk < 3
                sp = kk >= NKCH - 3
                nc.tensor.matmul(
                    q_ps[po : po + BS, :], xT_sb[:, kk, :], w_t[:, j, :QW],
                    start=st, stop=sp, tile_position=(0, po),
                )
                nc.tensor.matmul(
                    kv_ps[po : po + BS, :], xT_sb[:, kk, :], w_t[:, j, QW:],
                    start=st, stop=sp, tile_position=(0, po),
                )
        # DVE can read only one PSUM operand per instruction
        q_m = const.tile([BS, QW], f32, name="q_m")
        nc.vector.tensor_copy(out=q_m[:], in_=q_ps[0:BS, :])
        nc.vector.tensor_tensor(q_m[:], q_m[:], q_ps[BS : 2 * BS, :],
                                mybir.AluOpType.add)
        nc.vector.tensor_tensor(q_m[:], q_m[:], q_ps[2 * BS : 3 * BS, :],
                                mybir.AluOpType.add)
        kv_m = const.tile([BS, 2 * HD], f32, name="kv_m")
        nc.vector.tensor_copy(out=kv_m[:], in_=kv_ps[0:BS, :])
        nc.vector.tensor_tensor(kv_m[:], kv_m[:], kv_ps[BS : 2 * BS, :],
                                mybir.AluOpType.add)
        nc.vector.tensor_tensor(kv_m[:], kv_m[:], kv_ps[2 * BS : 3 * BS, :],
                                mybir.AluOpType.add)

        # ---- phase 2: rope, q/k transposes, new-token prep ----
        p2 = const
        # rope(q)
        q_sw = p2.tile([BS, QW], f32)
        q_ps3 = q_m[:].rearrange("p (i two) -> p i two", two=2)
        q_sw3 = q_sw[:].rearrange("p (i two) -> p i two", two=2)
        nc.vector.tensor_copy(out=q_sw3[:, :, 0], in_=q_ps3[:, :, 1])
        nc.vector.tensor_copy(out=q_sw3[:, :, 1], in_=q_ps3[:, :, 0])
        q_ro = p2.tile([BS, QW], f32)
        nc.vector.tensor_tensor(q_ro[:], q_m[:], cosq_sb[:], mybir.AluOpType.mult)
        nc.vector.tensor_tensor(q_sw[:], q_sw[:], sinq_sb[:], mybir.AluOpType.mult)
        nc.vector.tensor_tensor(q_ro[:], q_ro[:], q_sw[:], mybir.AluOpType.add)
        # rope(k) on kv_ps[:, :HD]
        k_sw = p2.tile([BS, HD], f32)
        k_ps3 = kv_m[:, :HD].rearrange("p (i two) -> p i two", two=2)
        k_sw3 = k_sw[:].rearrange("p (i two) -> p i two", two=2)
        nc.vector.tensor_copy(out=k_sw3[:, :, 0], in_=k_ps3[:, :, 1])
        nc.vector.tensor_copy(out=k_sw3[:, :, 1], in_=k_ps3[:, :, 0])
        k_ro = p2.tile([BS, HD], f32)
        nc.vector.tensor_tensor(
            k_ro[:], kv_m[:, :HD], cosk_sb[:], mybir.AluOpType.mult
        )
        nc.vector.tensor_tensor(k_sw[:], k_sw[:], sink_sb[:], mybir.AluOpType.mult)
        nc.vector.tensor_tensor(k_ro[:], k_ro[:], k_sw[:], mybir.AluOpType.add)
        # v_new (no rope)
        nc.vector.tensor_copy(out=vnew[:], in_=kv_m[:, HD:])

        # q^T assembly (b-major columns): qT_all[:, b, h] = q_ro[b, 128h + :]
        for h in range(HPC):
            ps_qt = psum_t2.tile([128, BS], f32, tag="tr")
            nc.tensor.transpose(
                ps_qt[:], q_ro[:, 128 * h : 128 * (h + 1)], iden_sb[:BS, :BS]
            )
            nc.vector.tensor_copy(out=qT_all[:, :, h], in_=ps_qt[:])
        # k_ro^T [128d, 32b] for the stale-column overwrite
        ps_kt = psum_t2.tile([128, BS], f32, tag="tr")
        nc.tensor.transpose(ps_kt[:], k_ro[:], iden_sb[:BS, :BS])
        nc.vector.tensor_copy(out=kroT[:], in_=ps_kt[:])

    # ---- phase 3a: QK scores, 4 batches per PSUM bank (col tiling) ----
    # Quarter qi, group g (4 batches): matmul l writes P_g[32l:32l+4] via
    # tile_position=(0,32l). Full-width drains P_g -> stage[:, g, :], then a
    # two-hop DMA fan-out packs partitions (l,h),g -> p = 16g+4l+h = 4b+h.
    # (Single-level partition patterns only: nested ones corrupt lowering.)
    scores = smpool.tile([128, NQ, QSEQ], f16)
    probs = smpool.tile([128, NQ * QSEQ], f16)
    den4 = smpool.tile([128, NQ], f32)
    stages = [smpool.tile([128, 8, QSEQ], f16, name=f"stage{i}") for i in (0, 1)]
    hops = [smpool.tile([16, 8, QSEQ], f16, name=f"hop{i}") for i in (0, 1)]

    def _drain(i, out, in_):
        # spread PSUM drains across the two PSUM-reading engines
        if i % 2 == 0:
            nc.vector.tensor_copy(out=out, in_=in_)
        else:
            nc.scalar.activation(
                out=out, in_=in_, func=mybir.ActivationFunctionType.Copy
            )

    # Bulk DMAs in one global FIFO on sync, priority order:
    # [consts | wqkv | kT8 q0,q1,q2 | wo (all 8) | V pairs 0-3 | kt16].
    # kt16 goes LAST so QK q3's hop fan-out runs with no bulk contention
    # behind it; V pairs 4-7 are emitted in phase B (their issues block on
    # PV consumption). The gpsimd ring carries only hops + final out DMAs.
    assert NQ8 == 3 and NQ - NQ8 == 1, "tuned for S=2048"
    kt_tiles = {}
    for qi in range(3):
        t = ktpool.tile([128, BS, QSEQ], f8, tag=f"kt{qi}", name=f"kt{qi}")
        for b0, b1 in ((0, 16), (16, 32)):
            nc.sync.dma_start(out=t[:, b0:b1, :], in_=kT8[:, qi, b0:b1, :])
        kt_tiles[qi] = t

    kt3a = ktpool.tile([128, BS // 2, QSEQ], bf16, tag="kt3a", name="kt3a")
    kt3b = ktpool.tile([128, BS // 2, QSEQ], bf16, tag="kt3b", name="kt3b")
    for half, t in ((0, kt3a), (1, kt3b)):
        base = half * (BS // 2)
        nc.sync.dma_start(out=t[:], in_=kT16[:, 0, base : base + BS // 2, :])
    NO = 8
    NW = DIM // NO
    wo_tiles = []
    for n in range(NO // 2):
        ns = slice(NW * n, NW * (n + 1))
        t = wopool.tile([128, HPC, NW], f8, tag="wo", bufs=4)
        nc.sync.dma_start(out=t[:], in_=wo[:, :, ns])
        wo_tiles.append(t)
    v_tiles = []
    for i in range(4):
        t = vpool.tile([128, 2, BS, HD], f8, tag="v", bufs=4)
        nc.sync.dma_start(out=t[:], in_=v8[:, 2 * i : 2 * i + 2, :, :])
        v_tiles.append(t)

    # probsT aliases the scores buffer (scores quarter qi is dead after its
    # exp; its transposes run inside the same quarter iteration)
    probsT = scores[:].rearrange("p q (c2 x) -> p (q c2) x", c2=4)
    E2 = smpool.tile([BS, 128], f16)

    def _quarter(qi, bank_of, psT):
        stage = stages[qi % 2]
        hop = hops[qi % 2]
        qs = QSEQ * qi
        qw = min(QSEQ, S - qs)
        if qi < 3:
            kt0 = kt_tiles[qi]
            ktA = ktB = kt0[:]
        else:
            kt0 = None
            ktA, ktB = kt3a, kt3b
        if qs <= S - 1 < qs + QSEQ:
            # overwrite the stale (appended) position's K with rope(k_new)
            if kt0 is not None:
                nc.vector.tensor_copy(out=kt0[:, :, S - 1 - qs], in_=kroT[:])
            else:
                nc.vector.tensor_copy(
                    out=ktA[:, :, S - 1 - qs], in_=kroT[:, : BS // 2]
                )
                nc.vector.tensor_copy(
                    out=ktB[:, :, S - 1 - qs], in_=kroT[:, BS // 2 :]
                )
        for g in range(8):
            P = bank_of(g)
            for l in range(4):
                b = 4 * g + l
                if kt0 is not None or b < BS // 2:
                    rhs = ktA[:, b, :qw]
                else:
                    rhs = ktB[:, b - BS // 2, :qw]
                nc.tensor.matmul(
                    P[32 * l : 32 * l + 4, :qw],
                    qT_all[:, b, :],
                    rhs,
                    start=True,
                    stop=True,
                    tile_position=(0, 32 * l),
                )
            _drain(g, stage[:, g, :qw], P[:, :qw])
        # two-hop fan-out on the (clear) gpsimd ring
        for l in range(4):
            nc.gpsimd.dma_start(
                out=hop[4 * l : 4 * l + 4, :, :qw],
                in_=stage[32 * l : 32 * l + 4, :, :qw],
            )
        # merged: dst 64-partition run splits naturally to (p:16 x4,
        # g':4 x1) giving partition p' = 64A + 4*(4l+h) + g'
        for A in range(2):
            nc.gpsimd.dma_start(
                out=scores[64 * A : 64 * A + 64, qi, :qw],
                in_=hop[:, 4 * A : 4 * A + 4, :qw],
            )
        # exp + per-quarter denominator; probs stay UNNORMALIZED -- attnT
        # is divided by den at the very end instead.
        if qw < QSEQ:
            nc.vector.memset(scores[:, qi, qw:], -30000.0)
        nc.scalar.activation(
            out=probs[:, qs : qs + QSEQ],
            in_=scores[:, qi, :],
            func=mybir.ActivationFunctionType.Exp,
            scale=SCALE,
        )
        # transpose this quarter's probs chunks (overwrites the quarter's
        # scores region, which exp has consumed)
        for c in range(4 * qi, 4 * qi + 4):
            cw = min(128, max(S - 128 * c, 0))
            if cw <= 0:
                continue
            ps_p = psT.tile([128, 128], f16, tag="ptr")
            nc.tensor.transpose(
                ps_p[:cw, :], probs[:, 128 * c : 128 * c + cw], iden16_sb[:]
            )
            if c % 2:
                nc.scalar.activation(
                    out=probsT[:cw, c, :], in_=ps_p[:cw, :],
                    func=mybir.ActivationFunctionType.Copy,
                )
            else:
                nc.vector.tensor_copy(out=probsT[:cw, c, :], in_=ps_p[:cw, :])
            if c == LC:
                # stale row of probsT must not touch the stale cached V
                nc.vector.tensor_tensor(
                    probsT[:, LC, :], probsT[:, LC, :],
                    rowmask_sb[:].to_broadcast([128, 128]),
                    mybir.AluOpType.mult,
                )

    # ---- phase A: quarters 0-2 ----
    with (
        tc.tile_pool(name="ps_qk", bufs=1, space="PSUM") as psQK,
        tc.tile_pool(name="ps_tr", bufs=3, space="PSUM") as psT,
    ):
        qk_ps = [psQK.tile([128, QSEQ], f32, name=f"qk{g}") for g in range(5)]
        for qi in range(NQ):
            _quarter(qi, lambda g, qi=qi: qk_ps[(8 * qi + g) % 5], psT)

    # ---- phase B: PV chunks 0-11 interleaved with QK q3, then PV 12-15 ----
    attnT = smpool.tile([128, BS, HPC], f16)  # [d, c2 = 32h + 4B + g']
    attnT_f = attnT[:].rearrange("p b h -> p (b h)")
    quad2 = []
    with tc.tile_pool(name="ps_pv", bufs=1, space="PSUM") as psPV:
        # V pairs 4-7: issues block on PV consuming the early pairs; sync
        # has nothing loop-critical behind these.
        for i in range(4, 8):
            t = vpool.tile([128, 2, BS, HD], f8, tag="v", bufs=4)
            nc.sync.dma_start(out=t[:], in_=v8[:, 2 * i : 2 * i + 2, :, :])
            v_tiles.append(t)
        pv_ps = [psPV.tile([128, 4 * HD], f32, name=f"pv{i}") for i in range(3)]
        for c in range(NCH):
            cw = min(128, S - 128 * c)
            v_v = v_tiles[c // 2][:].rearrange("p c2 b d -> p c2 (b d)")
            for Q in range(8):
                qo = 32 * (Q % 3)
                nc.tensor.matmul(
                    pv_ps[Q // 3][qo : qo + 4 * HPC, :],
                    probsT[:cw, c, 16 * Q : 16 * (Q + 1)],
                    v_v[:cw, c % 2, 512 * Q : 512 * (Q + 1)],
                    start=(c == 0),
                    stop=(c == NCH - 1),
                    tile_position=(0, qo),
                )
        for i in range(3):
            q_sb = smpool.tile([128, 4 * HD], f16, tag="quad", bufs=3,
                               name=f"quad{i}")
            _drain(i, q_sb[:], pv_ps[i][:])
            quad2.append(q_sb)

    # ---- denominator (single pass over all quarters) ----
    den = smpool.tile([128, 1], f32)
    nc.vector.tensor_reduce(
        out=den[:],
        in_=probs[:].rearrange("p (o s) -> p o s", o=1),
        axis=mybir.AxisListType.X,
        op=mybir.AluOpType.add,
    )
    inv = smpool.tile([128, 1], f32)
    nc.vector.reciprocal(inv[:], den[:])
    with (
        tc.tile_pool(name="ps_tr2", bufs=1, space="PSUM") as psT2,
        tc.tile_pool(name="ps_eb", bufs=1, space="PSUM") as psEb,
    ):
        # E2[b, c2] = p_new[c2] * (batch(c2) == b): rank-1 new-token update
        ps_pnr = psT2.tile([1, 128], f16, tag="pnr")
        nc.tensor.transpose(ps_pnr[:], probs[:, S - 1 : S], iden16_sb[:])
        pnr = smpool.tile([1, 128], f16)
        nc.vector.tensor_copy(
            out=pnr[:].rearrange("o (h A l g) -> o h A l g", A=2, l=4, h=4),
            in_=ps_pnr[:].rearrange("o (A l h g) -> o h A l g",
                                    A=2, l=4, h=4),
        )
        ps_eb = psEb.tile([BS, 128], f32)
        nc.tensor.matmul(ps_eb[:], ones32[:], pnr[:], start=True, stop=True)
        nc.vector.tensor_tensor(E2[:], ps_eb[:], iden4_sb[:], mybir.AluOpType.mult)

    with tc.tile_pool(name="ps_at", bufs=2, space="PSUM") as psAT:
        for Q in range(8):
            for j in range(HPC):
                ps_at = psAT.tile([128, 4 * HPC], f16, tag="at")
                qo = 32 * (Q % 3)
                nc.tensor.transpose(
                    ps_at[:],
                    quad2[Q // 3][qo : qo + 4 * HPC, 128 * j : 128 * (j + 1)],
                    iden16_sb[qo : qo + 4 * HPC, qo : qo + 4 * HPC],
                )
                # block Q cols are (4h + g'); row uses moving block j == g'.
                # attnT stores head-major: c2 = 32h + 4Q + g'.
                src = ps_at[:].rearrange("p (h g) -> p h g", h=4)[:, :, j]
                dst = attnT_f[:].rearrange(
                    "p (h B g) -> p h B g", h=4, B=8
                )[:, :, Q, j]
                eng = nc.scalar if j % 2 else nc.vector
                if eng is nc.scalar:
                    nc.scalar.activation(
                        out=dst, in_=src,
                        func=mybir.ActivationFunctionType.Copy,
                    )
                else:
                    nc.vector.tensor_copy(out=dst, in_=src)

    # ---- phase 3e: new-token rank-1 term, then deferred normalization ----
    with tc.tile_pool(name="ps_e2", bufs=1, space="PSUM") as psE2:
        ps_e2 = psE2.tile([128, 128], f32)
        nc.tensor.matmul(ps_e2[:], vnew[:], E2[:], start=True, stop=True)
        nc.vector.tensor_tensor(
            attnT_f[:], attnT_f[:], ps_e2[:], mybir.AluOpType.add
        )
        # attnT columns are (4b+h) = the partition order of inv: replicate
        # 1/den as rows via the PE (transpose then rank-1 with a ones row)
        # and divide attnT column-wise in one shot.
        ps_it = psE2.tile([1, 128], f32, name="ps_it")
        nc.tensor.transpose(ps_it[:], inv[:], iden_sb[:])
        invT = smpool.tile([1, 128], f16)
        nc.vector.tensor_copy(
            out=invT[:].rearrange("o (h A l g) -> o h A l g", A=2, l=4, h=4),
            in_=ps_it[:].rearrange("o (A l h g) -> o h A l g",
                                   A=2, l=4, h=4),
        )
        ps_ib = psE2.tile([128, 128], f32, name="ps_ib")
        nc.tensor.matmul(ps_ib[:], ones1r[:], invT[:], start=True, stop=True)
        nc.vector.tensor_tensor(
            attnT_f[:], attnT_f[:], ps_ib[:], mybir.AluOpType.mult
        )

    # ---- phase 4: output projection (wo fp8 chunks, descale on drain) ----
    for n in range(NO // 2, NO):
        ns = slice(NW * n, NW * (n + 1))
        t = wopool.tile([128, HPC, NW], f8, tag="wo", bufs=4)
        nc.gpsimd.dma_start(out=t[:], in_=wo[:, :, ns])
        wo_tiles.append(t)
    with tc.tile_pool(name="ps_o", bufs=1, space="PSUM") as psO:
        po_ps = [psO.tile([128, NW], f32, name=f"po{i}") for i in range(3)]
        for n in range(NO):
            ns = slice(NW * n, NW * (n + 1))
            po = 32 * (n % 3)
            P = po_ps[n % 3]
            for j in range(HPC):
                nc.tensor.matmul(
                    P[po : po + BS, :],
                    attnT_f[:, 32 * j : 32 * j + 32],
                    wo_tiles[n][:, j, :],
                    start=(j == 0),
                    stop=(j == HPC - 1),
                    tile_position=(0, po),
                )
            o_sb = outpool.tile([96, NW], f16, tag="ot", bufs=2)
            if n % 2 == 0:
                nc.scalar.activation(
                    out=o_sb[po : po + BS, :], in_=P[po : po + BS, :],
                    func=mybir.ActivationFunctionType.Copy,
                    scale=1.0 / WO_SCALE,
                )
            else:
                nc.vector.tensor_tensor(
                    o_sb[po : po + BS, :], P[po : po + BS, :],
                    inv64b[po : po + BS, :].to_broadcast([BS, NW]),
                    mybir.AluOpType.mult,
                )
            nc.gpsimd.dma_start(out=out[:, ns], in_=o_sb[po : po + BS, :])


@functools.lru_cache(maxsize=8)
def _built(start_pos, reps=1):
    return _build(start_pos, reps)


def _host_prep(x, wq, wk, wv, wo, cache_k, cache_v, freqs_cos, freqs_sin, start_pos):
    import ml_dtypes

    f8np = ml_dtypes.float8_e3m4
    bf16np = ml_dtypes.bfloat16
    S, NCH, NQ, NQ8 = _seq_split(start_pos)
    NQ16 = NQ - NQ8
    S8 = QSEQ * NQ8

    x = np.ascontiguousarray(np.asarray(x, dtype=np.float32)).reshape(BS, DIM)
    wq = np.asarray(wq, dtype=np.float32)
    wk = np.asarray(wk, dtype=np.float32)
    wv = np.asarray(wv, dtype=np.float32)
    wo = np.asarray(wo, dtype=np.float32)
    cache_k = np.asarray(cache_k, dtype=np.float32)
    cache_v = np.asarray(cache_v, dtype=np.float32)
    cos = np.asarray(freqs_cos, dtype=np.float32).reshape(HD // 2)
    sin = np.asarray(freqs_sin, dtype=np.float32).reshape(HD // 2)

    # x^T chunks: xT[p, c, b] = x[b, 128c + p]
    xT = np.ascontiguousarray(
        x.reshape(BS, DIM // 128, 128).transpose(2, 1, 0).astype(np.float16)
    )

    cosF = np.empty(HD, np.float32)
    cosF[0::2] = cos
    cosF[1::2] = cos
    sinF = np.empty(HD, np.float32)
    sinF[0::2] = -sin
    sinF[1::2] = sin
    cosq = np.ascontiguousarray(
        np.broadcast_to(np.tile(cosF, HPC), (BS, QW)).astype(np.float16))
    sinq = np.ascontiguousarray(
        np.broadcast_to(np.tile(sinF, HPC), (BS, QW)).astype(np.float16))
    cosk = np.ascontiguousarray(np.broadcast_to(cosF, (BS, HD)).astype(np.float16))
    sink = np.ascontiguousarray(np.broadcast_to(sinF, (BS, HD)).astype(np.float16))
    iden = np.eye(128, dtype=np.float32)
    iden16 = np.eye(128, dtype=np.float16)
    rowmask = np.ones((128, 1), dtype=np.float16)
    rowmask[start_pos % 128, 0] = 0.0
    # attnT/E2 column order c2 = 32h + 4B + g' (B = 4A + l, b = 4*(4A+g')+l)
    iden4 = np.zeros((BS, 128), np.float32)
    for b in range(BS):
        g, l = b // 4, b % 4
        A, gp = g // 4, g % 4
        for h in range(HPC):
            iden4[b, 32 * h + 4 * (4 * A + l) + gp] = 1.0

    # K^T quarters, padded to full QSEQ width
    SPAD = QSEQ * NQ
    in_maps = []
    for c in range(NCORES):
        wqkv_c = np.concatenate(
            [
                wq[:, QW * c : QW * (c + 1)],
                wk[:, HD * c : HD * (c + 1)],
                wv[:, HD * c : HD * (c + 1)],
            ],
            axis=1,
        )  # [DIM, 768]
        wqkv_c = np.ascontiguousarray(
            wqkv_c.reshape(DIM // 128, 128, QW + 2 * HD)
            .transpose(1, 0, 2)
            .astype(np.float16)
        )
        wo_c = np.ascontiguousarray(
            (wo[QW * c : QW * (c + 1), :] * WO_SCALE)
            .reshape(HPC, 128, DIM)
            .transpose(1, 0, 2)
            .astype(f8np)
        )
        ck = np.zeros((BS, SPAD, HD), np.float32)
        ck[:, :S] = cache_k[:, :S, c, :]
        # kq[d, q, b, s'] = ck[b, 512q + s', d]
        kT8_c = np.ascontiguousarray(
            ck[:, :max(S8, 1) if NQ8 else 1]
            .reshape(BS, max(NQ8, 1), -1, HD)
            .transpose(3, 1, 0, 2)
            .astype(f8np)
        ) if NQ8 else np.zeros((128, 1, BS, QSEQ), f8np)
        kT16_c = np.ascontiguousarray(
            ck[:, S8:]
            .reshape(BS, max(NQ16, 1), -1, HD)
            .transpose(3, 1, 0, 2)
            .astype(bf16np)
        ) if NQ16 else np.zeros((128, 1, BS, QSEQ), bf16np)
        # v8[p, ch, b_slot, d]: slot (B, j) holds batch 16*(B//4) + 4j + B%4
        vperm = np.empty(BS, np.int64)
        for Bb in range(8):
            for j in range(4):
                vperm[4 * Bb + j] = 16 * (Bb // 4) + 4 * j + (Bb % 4)
        v8_c = np.ascontiguousarray(
            cache_v[:, :, c, :]
            .reshape(BS, MAXSEQ // 128, 128, HD)
            .transpose(2, 1, 0, 3)[:, :, vperm, :]
            .astype(f8np)
        )
        in_maps.append(
            {
                "xT": xT,
                "wqkv": wqkv_c,
                "wo": wo_c,
                "kT8": kT8_c,
                "kT16": kT16_c,
                "v8": v8_c,
                "cosq": cosq,
                "sinq": sinq,
                "cosk": cosk,
                "sink": sink,
                "iden": iden,
                "iden16": iden16,
                "rowmask": rowmask,
                "iden4": iden4,
            }
        )
    return in_maps


def kernel(
    x,
    wq,
    wk,
    wv,
    wo,
    cache_k,
    cache_v,
    freqs_cos,
    freqs_sin,
    start_pos,
    _trace=False,
    **_unused,
):
    sp = int(start_pos)
    nc = _built(sp)
    in_maps = _host_prep(
        x, wq, wk, wv, wo, cache_k, cache_v, freqs_cos, freqs_sin, sp
    )
    res = run_bass_kernel_spmd(nc, in_maps, list(range(NCORES)), trace=_trace)
    # out rows come back permuted: row 4B+g' holds batch 16*(B//4)+4g'+(B%4)
    operm = np.empty(BS, np.int64)
    for Bb in range(8):
        for gp in range(4):
            operm[16 * (Bb // 4) + 4 * gp + (Bb % 4)] = 4 * Bb + gp
    acc = np.zeros((BS, DIM), np.float32)
    for i in range(NCORES):
        acc += np.asarray(res.results[i]["out"], dtype=np.float32)[operm]
    out = acc.reshape(BS, 1, DIM)
    if _trace:
        return out, res
    return out
